# revision 39
# baseline (speedup 1.0000x reference)
# Multi-head attention (b=2, n=2048, d_model=1024, 16 heads) on 8 NeuronCores.
#
# Sharding: core c = (batch b, head-group g) with b = c//4, g = c%4.
# Each core handles 1 batch element and 4 heads (256 channels), computing a
# partial output projection; the host sums the 4 group-partials per batch and
# adds b_O.
#
# Device layout (everything oriented so no transposes are needed):
#   xT   [D, N]      = x[b].T (bf16)             rhs of Q/K proj, lhsT of V
#   Q.T/K.T [2][128, N]  2 heads per 128-row tile (cs = h//2, rows (h%2)*64).
#   V    [N, CH] natural layout (+b_v), stored per-head with an appended
#        ones column: lhsT [m, 65] so the O-matmul's PSUM row 64 accumulates
#        the softmax denominators for free.
#   S.T  [m-slice, n] per head = K_h @ Q_h.T     (K=64 contraction)
#   E.T  = exp(S.T / 8) on ScalarE (scale folded into the activation), bf16
#   O.T+sums [65, n] = [V_h | 1].T @ E.T         (accumulate over m-slices)
#   Y.T  [D, N] = woT.T @ (O.T * recip(sums)), DVE-copied to fp16, DMA.
#
# Schedule: the attention loop (128 (chunk,head,ms) iterations) is paced by
# the ScalarE exp (~1.11us per [128,1024] tile).  All projection work is
# split into ~2-matmul "quanta" and drip-fed into the per-iteration slack by
# a deadline-driven scheduler, so the PE never idles and holds its 2.4 GHz
# p-state.  Softmax normalization is split: the reciprocal chain runs right
# after each head, but the (broadcast x multiply) into osb is deferred ~4
# iterations so the PE-queue broadcast matmul never stalls the stream.
#
# Matmul operands are bf16 (fp32 PSUM accumulation); fp32r measured ~3x
# slower on HW (cold-HAM equilibrium at ~630ns per 512-row matmul).

import ml_dtypes
import numpy as np

import concourse.bass as bass
import concourse.bacc as bacc
import concourse.tile as tile
from concourse import mybir
from concourse.bass_utils import run_bass_kernel_spmd

D = 1024  # d_model
N = 2048  # sequence length
B = 2  # batch
NHEADS = 16
DK = 64
NCORES = 8
GROUPS = 4  # head-groups across cores
HPG = NHEADS // GROUPS  # 4 heads per group
CH = HPG * DK  # 256 channels per group
KT = D // 128  # 8 contraction tiles for the projections
MS = N // 128  # 16 m-slices (key dim)
NCHUNK = 1024  # n-chunk width for the attention phase
NCHUNKS = N // NCHUNK

F32 = mybir.dt.float32
F16 = mybir.dt.float16
BF16 = mybir.dt.bfloat16


def _build_bass():
    nc = bacc.Bacc()

    xT_d = nc.dram_tensor("xT", [D, N], BF16, kind="ExternalInput")
    # wA: first-needed weight columns [wq_cs0 | wk_cs0]; wB: the rest
    # [wv | wq_cs1 | wk_cs1].  bqk: bq/bk as 4 columns of a 512B-row tile
    # (single efficient DMA instead of four 4B-descriptor ones).
    wA_d = nc.dram_tensor("wA", [D, 256], BF16, kind="ExternalInput")
    wB_d = nc.dram_tensor("wB", [D, 256], BF16, kind="ExternalInput")
    wv_d = nc.dram_tensor("wv", [D, 256], BF16, kind="ExternalInput")
    woT_d = nc.dram_tensor("woT", [CH, D], BF16, kind="ExternalInput")
    bqk_d = nc.dram_tensor("bqk", [128, 128], F32, kind="ExternalInput")
    bv_d = nc.dram_tensor("bv", [CH], F32, kind="ExternalInput")
    yT_d = nc.dram_tensor("yT", [D, N], F16, kind="ExternalOutput")

    with tile.TileContext(nc) as tc:
        with (
            tc.tile_pool(name="persist", bufs=1) as persist,
            tc.tile_pool(name="ph1", bufs=1) as ph1,
            tc.tile_pool(name="et_pool", bufs=4) as et_pool,
            tc.tile_pool(name="osb_pool", bufs=2) as osb_pool,
            tc.tile_pool(name="small", bufs=2) as small,
            tc.tile_pool(name="aux_ps", bufs=2, space="PSUM") as aux_ps,
            tc.tile_pool(name="st_ps", bufs=2, space="PSUM") as st_pool,
            tc.tile_pool(name="ot_ps", bufs=1, space="PSUM") as ot_pool,
        ):
            # ---- persistent tensors ----
            qt = [persist.tile([128, N], BF16, tag=f"qt{cs}", name=f"qt{cs}") for cs in range(CH // 128)]
            kt = [persist.tile([128, N], BF16, tag=f"kt{cs}", name=f"kt{cs}") for cs in range(CH // 128)]
            v4 = [persist.tile([128, HPG * 65], BF16, tag=f"v4_{ms}", name=f"v4_{ms}") for ms in range(MS)]
            wot = [persist.tile([128, D], BF16, tag=f"wot{cs}", name=f"wot{cs}") for cs in range(CH // 128)]

            # ---- input loads.  Emission order doubles as DMA-semaphore
            # allocation order (the sem pool is small and recycled FIFO), so
            # the critical-path loads (xt chunk 0, wA) are emitted FIRST;
            # later posts recycling their sems then wait on completions we
            # need anyway.  Posting is spread across sync/scalar/gpsimd.
            xt = [ph1.tile([128, N], BF16, tag=f"xt{k}", name=f"xt{k}") for k in range(KT)]
            for k in range(KT):
                nc.sync.dma_start(out=xt[k][:, 0:NCHUNK], in_=xT_d[k * 128 : (k + 1) * 128, 0:NCHUNK])
            wA, wB = [], []
            for k in range(KT):
                t = ph1.tile([128, 256], BF16, tag=f"wA{k}", name=f"wA{k}")
                nc.scalar.dma_start(out=t, in_=wA_d[k * 128 : (k + 1) * 128, :])
                wA.append(t)
            for k in range(KT):
                t = ph1.tile([128, 256], BF16, tag=f"wB{k}", name=f"wB{k}")
                nc.scalar.dma_start(out=t, in_=wB_d[k * 128 : (k + 1) * 128, :])
                wB.append(t)
            # wq[k][cs], wk[k][cs] views into the packed tiles
            wq = [(wA[k][:, 0:128], wB[k][:, 0:128]) for k in range(KT)]
            wk = [(wA[k][:, 128:256], wB[k][:, 128:256]) for k in range(KT)]
            # gpsimd queue: constants, V weights (needed from iter 0), then
            # late-needed weights/biases
            for ms in range(MS):
                v4v = v4[ms].rearrange("p (h c) -> p h c", c=65)
                nc.gpsimd.memset(v4v[:, :, 64:65], 1.0)
            bqk = ph1.tile([128, 128], F32, tag="bqk", name="bqk")
            nc.gpsimd.dma_start(out=bqk, in_=bqk_d[:, :])
            bq_t = [bqk[:, 0:1], bqk[:, 1:2]]
            bk_t = [bqk[:, 2:3], bqk[:, 3:4]]
            bvb = ph1.tile([128, CH], F32, tag="bvb", name="bvb")
            bv_ap = bv_d[None, :]
            nc.gpsimd.dma_start(
                out=bvb,
                in_=bass.AP(tensor=bv_ap.tensor, offset=bv_ap.offset, ap=[[0, 128]] + list(bv_ap.ap[1:])),
            )
            wv = []
            for k in range(KT):
                t = ph1.tile([128, 256], BF16, tag=f"wv{k}", name=f"wv{k}")
                nc.gpsimd.dma_start(out=t, in_=wv_d[k * 128 : (k + 1) * 128, :])
                wv.append(t)
            for cs in range(CH // 128):
                nc.gpsimd.dma_start(out=wot[cs], in_=woT_d[cs * 128 : (cs + 1) * 128, :])
            # xt columns 1024:2048 last: first needed by V(8)/k-n1024 fillers
            # around iter 5, long after the first-gate loads above.
            for k in range(KT):
                nc.gpsimd.dma_start(out=xt[k][:, NCHUNK:N], in_=xT_d[k * 128 : (k + 1) * 128, NCHUNK:N])

            osb_tiles = {}

            # ---- chain emitters, split into ~2-matmul quanta ----
            def qk_chain_quanta(isq, cs, n0):
                dst, w, bias = (qt, wq, bq_t) if isq else (kt, wk, bk_t)
                st_ = {}

                def step(k0, k1):
                    if k0 == 0:
                        st_["ps"] = aux_ps.tile([128, 512], F32, tag="aux", name="aux_ps_t")
                    ps = st_["ps"]
                    for k in range(k0, k1):
                        nc.tensor.matmul(
                            ps,
                            w[k][cs],
                            xt[k][:, n0 : n0 + 512],
                            start=(k == 0),
                            stop=(k == KT - 1),
                        )
                    if k1 == KT:
                        nc.vector.tensor_scalar_add(
                            out=dst[cs][:, n0 : n0 + 512], in0=ps, scalar1=bias[cs]
                        )

                return [lambda a=a, b=b: step(a, b) for a, b in ((0, 2), (2, 4), (4, 6), (6, 8))]

            def v_chain_quanta(ms):
                st_ = {}

                def step(k0, k1):
                    if k0 == 0:
                        st_["ps"] = aux_ps.tile([128, 512], F32, tag="aux", name="aux_ps_t")
                    ps = st_["ps"]
                    for k in range(k0, k1):
                        nc.tensor.matmul(
                            ps[:, 0:CH],
                            xt[k][:, ms * 128 : (ms + 1) * 128],
                            wv[k],
                            start=(k == 0),
                            stop=(k == KT - 1),
                        )
                    if k1 == KT:
                        v4v = v4[ms].rearrange("p (h c) -> p h c", c=65)
                        nc.vector.tensor_add(
                            out=v4v[:, :, 0:64],
                            in0=ps[:, 0:CH].rearrange("p (h c) -> p h c", c=64),
                            in1=bvb.rearrange("p (h c) -> p h c", c=64),
                        )

                return [lambda a=a, b=b: step(a, b) for a, b in ((0, 2), (2, 4), (4, 6), (6, 8))]

            def emit_f(chunk, msl, j, on_scalar=False, on_st_pool=False):
                n0 = chunk * NCHUNK
                osb = osb_tiles[chunk]
                if on_st_pool:  # epilogue: st_pool is idle, fresh WAR history
                    yp = st_pool.tile([128, NCHUNK], F32, tag="st", name="st_yt_t")[:, 0:512]
                else:
                    yp = aux_ps.tile([128, 512], F32, tag="aux", name="aux_yt_t")
                for cs in range(CH // 128):
                    nc.tensor.matmul(
                        yp,
                        wot[cs][:, msl * 128 : (msl + 1) * 128],
                        osb[cs][:, j : j + 512],
                        start=(cs == 0),
                        stop=(cs == CH // 128 - 1),
                    )
                ysb = small.tile([128, 512], F16, tag="ysb", name="ysb_t", bufs=4)
                if on_scalar:  # epilogue: ScalarE is idle, halve the copy load
                    nc.scalar.copy(out=ysb, in_=yp)
                else:
                    nc.vector.tensor_copy(out=ysb, in_=yp)
                nc.sync.dma_start(
                    out=yT_d[msl * 128 : (msl + 1) * 128, n0 + j : n0 + j + 512],
                    in_=ysb,
                )

            # softmax normalization, entirely off the PE: drain ot, reshuffle
            # the denominator row to [128, 8] via a tiny DMA, reciprocal,
            # flatten back, partition-broadcast on the (idle) GpSimd, and
            # scale the O.T rows into osb on the DVE.
            def norm_chain(chunk, h, oraw):
                # [8, 128] staging keeps both reshuffle DMAs at 8 fat
                # descriptors instead of 128 tiny ones (~0.9us vs ~2.4us).
                cs, r0 = h // 2, (h % 2) * 64
                rcin = small.tile([8, 128], F32, tag="rcin", name="rcin_t")
                nc.gpsimd.dma_start(out=rcin, in_=oraw[64:65, :])
                rc = small.tile([8, 128], F32, tag="rc", name="rc_t")
                nc.vector.reciprocal(out=rc, in_=rcin)
                rflat = small.tile([1, NCHUNK], F32, tag="rflat", name="rflat_t")
                nc.gpsimd.dma_start(out=rflat, in_=rc)
                rb = small.tile([128, NCHUNK], F32, tag="rb", name="rb_t")
                nc.gpsimd.partition_broadcast(rb, rflat)
                nc.vector.tensor_mul(
                    out=osb_tiles[chunk][cs][r0 : r0 + 64, :],
                    in0=oraw[0:64, :],
                    in1=rb[0:64, :],
                )

            def normalize(chunk, h, ot):
                oraw = small.tile([65, NCHUNK], F32, tag="oraw", name="oraw_t")
                nc.vector.tensor_copy(out=oraw, in_=ot)
                norm_chain(chunk, h, oraw)

            # Final head's normalization, split so the reserved output-
            # projection groups can be emitted in between: part A drains ot
            # on the (now idle) ScalarE; part B runs the reciprocal chain.
            # Nothing in the reserved groups then waits on the chain through
            # the in-order DVE/scalar queues.
            def normalize_last_a(ot):
                oraw = small.tile([65, NCHUNK], F32, tag="oraw", name="oraw_t")
                nc.scalar.copy(out=oraw, in_=ot)
                return oraw

            # ---- quantum schedule.  Chains are kept CONTIGUOUS in pop
            # order (only consecutive chains ever co-occupy the 2-buffer
            # aux PSUM pool); each chain gets a start deadline and its
            # quanta inherit start+qi.  Negative deadlines pop at iter 0.
            chains = []  # (start_deadline, ready_iter, [quanta...])
            for ms in range(0, MS):  # V(ms) must close before O at iter ms
                chains.append((ms - 3, 0, v_chain_quanta(ms)))
            for i, n0 in enumerate((512, 1024, 1536)):  # kt cs0, st @ iter 4i+4
                chains.append((4 * i + 1, 0, qk_chain_quanta(False, 0, n0)))
            for i, n0 in enumerate((0, 512)):  # qt cs1 chunk0 (h2c0 @ iter 32)
                chains.append((24 + 2 * i, 0, qk_chain_quanta(True, 1, n0)))
            for i, n0 in enumerate((0, 512, 1024, 1536)):  # kt cs1 (h2c0)
                rdy = 12 if n0 >= 1024 else 0
                chains.append((28 + 4 * i, rdy, qk_chain_quanta(False, 1, n0)))
            for i, n0 in enumerate((1024, 1536)):  # qt cs0 chunk1 (h0c1 @ 64)
                chains.append((48 + 4 * i, 12, qk_chain_quanta(True, 0, n0)))
            for i, n0 in enumerate((1024, 1536)):  # qt cs1 chunk1 (h2c1 @ 96)
                chains.append((58 + 4 * i, 12, qk_chain_quanta(True, 1, n0)))
            # output projection for chunk 0: fillers once chunk-0 osb is
            # fully normalized (~iter 68); hold back the last 10 groups to
            # keep the PE warm through the final reciprocal chain.
            fgroups = [(msl, j) for msl in range(D // 128) for j in range(0, NCHUNK, 512)]
            for i, (msl, j) in enumerate(fgroups[:6]):
                chains.append((69 + 4 * i, 69, [lambda m=msl, jj=j: emit_f(0, m, jj)]))
            tail_reserve = fgroups[6:]
            chains.sort(key=lambda c: c[0])
            quanta = [
                (start + qi, rdy, fn)
                for start, rdy, qs in chains
                for qi, fn in enumerate(qs)
            ]

            def run_sched(t):
                popped = 0
                while quanta:
                    dl, rdy, fn = quanta[0]
                    due = dl <= t or any(q[0] <= t for q in quanta[1:6])
                    if due or (popped == 0 and rdy <= t):
                        quanta.pop(0)
                        fn()
                        popped += 1
                    else:
                        break

            # ---- prelude: what (h0, c0) iter-0 needs, in DMA-arrival order
            for fn in qk_chain_quanta(True, 0, 0):
                fn()
            for fn in qk_chain_quanta(False, 0, 0):
                fn()
            for fn in qk_chain_quanta(True, 0, 512):
                fn()

            # ---- attention + output projection.  The scores matmuls are
            # software-pipelined one iteration ahead of the exp so the
            # Scalar engine is never gated by the PE queue: PE order per
            # iter is [fillers, st(i+1), O(i)], with exp(i) already issued.
            seq = [
                (c, h, ms)
                for c in range(NCHUNKS)
                for h in range(HPG)
                for ms in range(MS)
            ]

            def emit_st(idx):
                c, h, ms = seq[idx]
                cs, r0 = h // 2, (h % 2) * 64
                st = st_pool.tile([128, NCHUNK], F32, tag="st", name="st_t")
                for j in range(0, NCHUNK, 512):
                    nc.tensor.matmul(
                        st[:, j : j + 512],
                        kt[cs][r0 : r0 + 64, ms * 128 : (ms + 1) * 128],
                        qt[cs][r0 : r0 + 64, c * NCHUNK + j : c * NCHUNK + j + 512],
                        start=True,
                        stop=True,
                    )
                return st

            sts = {0: emit_st(0)}
            ot = None
            for it, (chunk, h, ms) in enumerate(seq):
                if ms == 0:
                    if h == 0:
                        osb_tiles[chunk] = [
                            osb_pool.tile([128, NCHUNK], BF16, tag=f"osb{cs}", name=f"osb{cs}")
                            for cs in range(CH // 128)
                        ]
                    ot = ot_pool.tile([65, NCHUNK], F32, tag="ot", name="ot_t")
                et = et_pool.tile([128, NCHUNK], BF16, tag="et", name="et_t")
                nc.scalar.activation(
                    out=et,
                    in_=sts.pop(it),
                    func=mybir.ActivationFunctionType.Exp,
                    scale=float(1.0 / np.sqrt(DK)),
                )
                if it + 1 < len(seq):
                    sts[it + 1] = emit_st(it + 1)
                run_sched(it)
                lhsT = v4[ms][:, h * 65 : (h + 1) * 65]
                for j in range(0, NCHUNK, 512):
                    nc.tensor.matmul(
                        ot[:, j : j + 512],
                        lhsT,
                        et[:, j : j + 512],
                        start=(ms == 0),
                        stop=(ms == MS - 1),
                    )
                if ms == MS - 1 and it + 1 < len(seq):
                    normalize(chunk, h, ot)
            # ---- epilogue: drain ot on the idle ScalarE, run the reserved
            # chunk-0 groups (copies alternating DVE/ScalarE) while the
            # reciprocal chain completes, then chunk 1's projection.
            assert not quanta, len(quanta)
            oraw_last = normalize_last_a(ot)
            for i, (msl, j) in enumerate(tail_reserve):
                emit_f(0, msl, j, on_scalar=(i % 2 == 1), on_st_pool=True)
            norm_chain(NCHUNKS - 1, HPG - 1, oraw_last)
            for i in range(D // 128 * 2):
                msl, j = divmod(i, 2)
                emit_f(1, msl, j * 512, on_scalar=(i % 2 == 1), on_st_pool=True)
    nc.compile()
    return nc


_NC = None


def _get_nc():
    global _NC
    if _NC is None:
        _NC = _build_bass()
    return _NC


def build_in_maps(inputs):
    x = np.asarray(inputs["x"], dtype=np.float32)
    W_Q = np.asarray(inputs["W_Q"], dtype=np.float32)
    W_K = np.asarray(inputs["W_K"], dtype=np.float32)
    W_V = np.asarray(inputs["W_V"], dtype=np.float32)
    W_O = np.asarray(inputs["W_O"], dtype=np.float32)
    b_Q = np.asarray(inputs["b_Q"], dtype=np.float32)
    b_K = np.asarray(inputs["b_K"], dtype=np.float32)
    b_V = np.asarray(inputs["b_V"], dtype=np.float32)

    in_maps = []
    for c in range(NCORES):
        b, g = divmod(c, GROUPS)
        lo = g * CH
        sl = slice(lo, lo + CH)
        wA = np.concatenate(
            [W_Q[lo : lo + 128, :].T, W_K[lo : lo + 128, :].T], axis=1
        )
        wB = np.concatenate(
            [W_Q[lo + 128 : lo + 256, :].T, W_K[lo + 128 : lo + 256, :].T],
            axis=1,
        )
        bqk = np.zeros((128, 128), dtype=np.float32)
        bqk[:, 0] = b_Q[lo : lo + 128]
        bqk[:, 1] = b_Q[lo + 128 : lo + 256]
        bqk[:, 2] = b_K[lo : lo + 128]
        bqk[:, 3] = b_K[lo + 128 : lo + 256]
        in_maps.append(
            {
                "xT": np.ascontiguousarray(x[b].T.astype(ml_dtypes.bfloat16)),
                "wA": np.ascontiguousarray(wA.astype(ml_dtypes.bfloat16)),
                "wB": np.ascontiguousarray(wB.astype(ml_dtypes.bfloat16)),
                "wv": np.ascontiguousarray(W_V[sl, :].T.astype(ml_dtypes.bfloat16)),
                "woT": np.ascontiguousarray(W_O[:, sl].T.astype(ml_dtypes.bfloat16)),
                "bqk": bqk,
                "bv": np.ascontiguousarray(b_V[sl]),
            }
        )
    return in_maps


def kernel(**inputs):
    in_maps = build_in_maps(inputs)
    nc = _get_nc()
    res = run_bass_kernel_spmd(nc, in_maps, core_ids=list(range(NCORES)))

    b_O = np.asarray(inputs["b_O"], dtype=np.float32)
    out = np.zeros((B, N, D), dtype=np.float32)
    for c in range(NCORES):
        b = c // GROUPS
        out[b] += res.results[c]["yT"].T.astype(np.float32)
    out += b_O
    return out


# revision 45
# speedup vs baseline: 1.0051x; 1.0051x over previous
# Multi-head attention (b=2, n=2048, d_model=1024, 16 heads) on 8 NeuronCores.
#
# Sharding: core c = (batch b, head-group g) with b = c//4, g = c%4.
# Each core handles 1 batch element and 4 heads (256 channels), computing a
# partial output projection; the host sums the 4 group-partials per batch and
# adds b_O.
#
# Device layout (everything oriented so no transposes are needed):
#   xT   [D, N]      = x[b].T (bf16)             rhs of Q/K proj, lhsT of V
#   Q.T/K.T [2][128, N]  2 heads per 128-row tile (cs = h//2, rows (h%2)*64).
#   V    [N, CH] natural layout (+b_v), stored per-head with an appended
#        ones column: lhsT [m, 65] so the O-matmul's PSUM row 64 accumulates
#        the softmax denominators for free.
#   S.T  [m-slice, n] per head = K_h @ Q_h.T     (K=64 contraction)
#   E.T  = exp(S.T / 8) on ScalarE (scale folded into the activation), bf16
#   O.T+sums [65, n] = [V_h | 1].T @ E.T         (accumulate over m-slices)
#   Y.T  [D, N] = woT.T @ (O.T * recip(sums)), DVE-copied to fp16, DMA.
#
# Schedule: the attention loop (128 (chunk,head,ms) iterations) is paced by
# the ScalarE exp (~1.11us per [128,1024] tile).  All projection work is
# split into ~2-matmul "quanta" and drip-fed into the per-iteration slack by
# a deadline-driven scheduler, so the PE never idles and holds its 2.4 GHz
# p-state.  Softmax normalization is split: the reciprocal chain runs right
# after each head, but the (broadcast x multiply) into osb is deferred ~4
# iterations so the PE-queue broadcast matmul never stalls the stream.
#
# Matmul operands are bf16 (fp32 PSUM accumulation); fp32r measured ~3x
# slower on HW (cold-HAM equilibrium at ~630ns per 512-row matmul).

import ml_dtypes
import numpy as np

import concourse.bass as bass
import concourse.bacc as bacc
import concourse.tile as tile
from concourse import mybir
from concourse.bass_utils import run_bass_kernel_spmd

D = 1024  # d_model
N = 2048  # sequence length
B = 2  # batch
NHEADS = 16
DK = 64
NCORES = 8
GROUPS = 4  # head-groups across cores
HPG = NHEADS // GROUPS  # 4 heads per group
CH = HPG * DK  # 256 channels per group
KT = D // 128  # 8 contraction tiles for the projections
MS = N // 128  # 16 m-slices (key dim)
NCHUNK = 1024  # n-chunk width for the attention phase
NCHUNKS = N // NCHUNK

F32 = mybir.dt.float32
F16 = mybir.dt.float16
BF16 = mybir.dt.bfloat16


def _build_bass():
    nc = bacc.Bacc()

    xT_d = nc.dram_tensor("xT", [D, N], BF16, kind="ExternalInput")
    # wA: first-needed weight columns [wq_cs0 | wk_cs0]; wB: the rest
    # [wv | wq_cs1 | wk_cs1].  bqk: bq/bk as 4 columns of a 512B-row tile
    # (single efficient DMA instead of four 4B-descriptor ones).
    wA_d = nc.dram_tensor("wA", [D, 256], BF16, kind="ExternalInput")
    wB_d = nc.dram_tensor("wB", [D, 256], BF16, kind="ExternalInput")
    wv_d = nc.dram_tensor("wv", [D, 256], BF16, kind="ExternalInput")
    woT_d = nc.dram_tensor("woT", [CH, D], BF16, kind="ExternalInput")
    bqk_d = nc.dram_tensor("bqk", [128, 128], F32, kind="ExternalInput")
    bv_d = nc.dram_tensor("bv", [CH], F32, kind="ExternalInput")
    yT_d = nc.dram_tensor("yT", [D, N], F16, kind="ExternalOutput")

    with tile.TileContext(nc) as tc:
        with (
            tc.tile_pool(name="persist", bufs=1) as persist,
            tc.tile_pool(name="ph1", bufs=1) as ph1,
            tc.tile_pool(name="et_pool", bufs=4) as et_pool,
            tc.tile_pool(name="osb_pool", bufs=2) as osb_pool,
            tc.tile_pool(name="small", bufs=2) as small,
            tc.tile_pool(name="aux_ps", bufs=2, space="PSUM") as aux_ps,
            tc.tile_pool(name="st_ps", bufs=2, space="PSUM") as st_pool,
            tc.tile_pool(name="ot_ps", bufs=1, space="PSUM") as ot_pool,
        ):
            # ---- persistent tensors ----
            qt = [persist.tile([128, N], BF16, tag=f"qt{cs}", name=f"qt{cs}") for cs in range(CH // 128)]
            kt = [persist.tile([128, N], BF16, tag=f"kt{cs}", name=f"kt{cs}") for cs in range(CH // 128)]
            v4 = [persist.tile([128, HPG * 65], BF16, tag=f"v4_{ms}", name=f"v4_{ms}") for ms in range(MS)]
            wot = [persist.tile([128, D], BF16, tag=f"wot{cs}", name=f"wot{cs}") for cs in range(CH // 128)]

            # ---- input loads.  Emission order doubles as DMA-semaphore
            # allocation order (the sem pool is small and recycled FIFO), so
            # the critical-path loads (xt chunk 0, wA) are emitted FIRST;
            # later posts recycling their sems then wait on completions we
            # need anyway.  Posting is spread across sync/scalar/gpsimd.
            xt = [ph1.tile([128, N], BF16, tag=f"xt{k}", name=f"xt{k}") for k in range(KT)]
            for k in range(KT):
                nc.sync.dma_start(out=xt[k][:, 0:NCHUNK], in_=xT_d[k * 128 : (k + 1) * 128, 0:NCHUNK])
            wA, wB = [], []
            for k in range(KT):
                t = ph1.tile([128, 256], BF16, tag=f"wA{k}", name=f"wA{k}")
                nc.scalar.dma_start(out=t, in_=wA_d[k * 128 : (k + 1) * 128, :])
                wA.append(t)
            for k in range(KT):
                t = ph1.tile([128, 256], BF16, tag=f"wB{k}", name=f"wB{k}")
                nc.scalar.dma_start(out=t, in_=wB_d[k * 128 : (k + 1) * 128, :])
                wB.append(t)
            for k in range(KT):
                nc.sync.dma_start(out=xt[k][:, NCHUNK:N], in_=xT_d[k * 128 : (k + 1) * 128, NCHUNK:N])
            # wq[k][cs], wk[k][cs] views into the packed tiles
            wq = [(wA[k][:, 0:128], wB[k][:, 0:128]) for k in range(KT)]
            wk = [(wA[k][:, 128:256], wB[k][:, 128:256]) for k in range(KT)]
            # gpsimd queue: constants, V weights (needed from iter 0), then
            # late-needed weights/biases
            for ms in range(MS):
                v4v = v4[ms].rearrange("p (h c) -> p h c", c=65)
                nc.gpsimd.memset(v4v[:, :, 64:65], 1.0)
            bqk = ph1.tile([128, 128], F32, tag="bqk", name="bqk")
            nc.gpsimd.dma_start(out=bqk, in_=bqk_d[:, :])
            bq_t = [bqk[:, 0:1], bqk[:, 1:2]]
            bk_t = [bqk[:, 2:3], bqk[:, 3:4]]
            bvb = ph1.tile([128, CH], F32, tag="bvb", name="bvb")
            bv_ap = bv_d[None, :]
            nc.gpsimd.dma_start(
                out=bvb,
                in_=bass.AP(tensor=bv_ap.tensor, offset=bv_ap.offset, ap=[[0, 128]] + list(bv_ap.ap[1:])),
            )
            wv = []
            for k in range(KT):
                t = ph1.tile([128, 256], BF16, tag=f"wv{k}", name=f"wv{k}")
                nc.gpsimd.dma_start(out=t, in_=wv_d[k * 128 : (k + 1) * 128, :])
                wv.append(t)
            for cs in range(CH // 128):
                nc.gpsimd.dma_start(out=wot[cs], in_=woT_d[cs * 128 : (cs + 1) * 128, :])

            osb_tiles = {}

            # ---- chain emitters, split into ~2-matmul quanta ----
            def qk_chain_quanta(isq, cs, n0):
                dst, w, bias = (qt, wq, bq_t) if isq else (kt, wk, bk_t)
                st_ = {}

                def step(k0, k1):
                    if k0 == 0:
                        st_["ps"] = aux_ps.tile([128, 512], F32, tag="aux", name="aux_ps_t")
                    ps = st_["ps"]
                    for k in range(k0, k1):
                        nc.tensor.matmul(
                            ps,
                            w[k][cs],
                            xt[k][:, n0 : n0 + 512],
                            start=(k == 0),
                            stop=(k == KT - 1),
                        )
                    if k1 == KT:
                        nc.vector.tensor_scalar_add(
                            out=dst[cs][:, n0 : n0 + 512], in0=ps, scalar1=bias[cs]
                        )

                return [lambda a=a, b=b: step(a, b) for a, b in ((0, 2), (2, 4), (4, 6), (6, 8))]

            def v_chain_quanta(ms):
                st_ = {}

                def step(k0, k1):
                    if k0 == 0:
                        st_["ps"] = aux_ps.tile([128, 512], F32, tag="aux", name="aux_ps_t")
                    ps = st_["ps"]
                    for k in range(k0, k1):
                        nc.tensor.matmul(
                            ps[:, 0:CH],
                            xt[k][:, ms * 128 : (ms + 1) * 128],
                            wv[k],
                            start=(k == 0),
                            stop=(k == KT - 1),
                        )
                    if k1 == KT:
                        v4v = v4[ms].rearrange("p (h c) -> p h c", c=65)
                        nc.vector.tensor_add(
                            out=v4v[:, :, 0:64],
                            in0=ps[:, 0:CH].rearrange("p (h c) -> p h c", c=64),
                            in1=bvb.rearrange("p (h c) -> p h c", c=64),
                        )

                return [lambda a=a, b=b: step(a, b) for a, b in ((0, 2), (2, 4), (4, 6), (6, 8))]

            def emit_f(chunk, msl, j, epi=None):
                # epi: epilogue slot index — rotates PSUM over the idle
                # st_pool as well as aux, alternates the drain copy between
                # ScalarE and DVE, and the yT DMA between sync and gpsimd.
                n0 = chunk * NCHUNK
                osb = osb_tiles[chunk]
                if epi is not None and epi % 2 == 0:
                    yp = st_pool.tile([128, NCHUNK], F32, tag="st", name="st_yt_t")[:, 0:512]
                else:
                    yp = aux_ps.tile([128, 512], F32, tag="aux", name="aux_yt_t")
                for cs in range(CH // 128):
                    nc.tensor.matmul(
                        yp,
                        wot[cs][:, msl * 128 : (msl + 1) * 128],
                        osb[cs][:, j : j + 512],
                        start=(cs == 0),
                        stop=(cs == CH // 128 - 1),
                    )
                ysb = small.tile([128, 512], F16, tag="ysb", name="ysb_t", bufs=4)
                if epi is not None and epi % 2 == 1:
                    nc.scalar.copy(out=ysb, in_=yp)
                else:
                    nc.vector.tensor_copy(out=ysb, in_=yp)
                # gpsimd carries odd c1-group DMAs only (epi >= 10): the
                # reciprocal chain's DMAs sit on gpsimd between the tail and
                # c1 groups and must not queue behind tail yT writes.
                dma_eng = nc.gpsimd if (epi is not None and epi >= 10 and epi % 2 == 1) else nc.sync
                dma_eng.dma_start(
                    out=yT_d[msl * 128 : (msl + 1) * 128, n0 + j : n0 + j + 512],
                    in_=ysb,
                )

            # softmax normalization, entirely off the PE: drain ot, reshuffle
            # the denominator row to [128, 8] via a tiny DMA, reciprocal,
            # flatten back, partition-broadcast on the (idle) GpSimd, and
            # scale the O.T rows into osb on the DVE.
            def norm_chain(chunk, h, oraw):
                # [8, 128] staging keeps both reshuffle DMAs at 8 fat
                # descriptors instead of 128 tiny ones (~0.9us vs ~2.4us).
                cs, r0 = h // 2, (h % 2) * 64
                rcin = small.tile([8, 128], F32, tag="rcin", name="rcin_t")
                nc.gpsimd.dma_start(out=rcin, in_=oraw[64:65, :])
                rc = small.tile([8, 128], F32, tag="rc", name="rc_t")
                nc.vector.reciprocal(out=rc, in_=rcin)
                rflat = small.tile([1, NCHUNK], F32, tag="rflat", name="rflat_t")
                nc.gpsimd.dma_start(out=rflat, in_=rc)
                rb = small.tile([128, NCHUNK], F32, tag="rb", name="rb_t")
                nc.gpsimd.partition_broadcast(rb, rflat)
                nc.vector.tensor_mul(
                    out=osb_tiles[chunk][cs][r0 : r0 + 64, :],
                    in0=oraw[0:64, :],
                    in1=rb[0:64, :],
                )

            def normalize(chunk, h, ot):
                oraw = small.tile([65, NCHUNK], F32, tag="oraw", name="oraw_t")
                nc.vector.tensor_copy(out=oraw, in_=ot)
                norm_chain(chunk, h, oraw)

            # Final head's normalization, split so the reserved output-
            # projection groups can be emitted in between: part A drains ot
            # on the (now idle) ScalarE; part B runs the reciprocal chain.
            # Nothing in the reserved groups then waits on the chain through
            # the in-order DVE/scalar queues.
            def normalize_last_a(ot):
                oraw = small.tile([65, NCHUNK], F32, tag="oraw", name="oraw_t")
                nc.scalar.copy(out=oraw, in_=ot)
                return oraw

            # ---- quantum schedule.  Chains are kept CONTIGUOUS in pop
            # order (only consecutive chains ever co-occupy the 2-buffer
            # aux PSUM pool); each chain gets a start deadline and its
            # quanta inherit start+qi.  Negative deadlines pop at iter 0.
            chains = []  # (start_deadline, ready_iter, [quanta...])
            for ms in range(0, MS):  # V(ms) must close before O at iter ms
                chains.append((ms - 3, 0, v_chain_quanta(ms)))
            for i, n0 in enumerate((512, 1024, 1536)):  # kt cs0, st @ iter 4i+4
                chains.append((4 * i + 1, 0, qk_chain_quanta(False, 0, n0)))
            for i, n0 in enumerate((0, 512)):  # qt cs1 chunk0 (h2c0 @ iter 32)
                chains.append((24 + 2 * i, 0, qk_chain_quanta(True, 1, n0)))
            for i, n0 in enumerate((0, 512, 1024, 1536)):  # kt cs1 (h2c0)
                rdy = 12 if n0 >= 1024 else 0
                chains.append((28 + 4 * i, rdy, qk_chain_quanta(False, 1, n0)))
            for i, n0 in enumerate((1024, 1536)):  # qt cs0 chunk1 (h0c1 @ 64)
                chains.append((48 + 4 * i, 12, qk_chain_quanta(True, 0, n0)))
            for i, n0 in enumerate((1024, 1536)):  # qt cs1 chunk1 (h2c1 @ 96)
                chains.append((58 + 4 * i, 12, qk_chain_quanta(True, 1, n0)))
            # output projection for chunk 0: fillers once chunk-0 osb is
            # fully normalized (~iter 68); hold back the last 10 groups to
            # keep the PE warm through the final reciprocal chain.
            fgroups = [(msl, j) for msl in range(D // 128) for j in range(0, NCHUNK, 512)]
            for i, (msl, j) in enumerate(fgroups[:6]):
                chains.append((69 + 4 * i, 69, [lambda m=msl, jj=j: emit_f(0, m, jj)]))
            tail_reserve = fgroups[6:]
            chains.sort(key=lambda c: c[0])
            quanta = [
                (start + qi, rdy, fn)
                for start, rdy, qs in chains
                for qi, fn in enumerate(qs)
            ]

            def run_sched(t):
                popped = 0
                while quanta:
                    dl, rdy, fn = quanta[0]
                    due = dl <= t or any(q[0] <= t for q in quanta[1:6])
                    if due or (popped == 0 and rdy <= t):
                        quanta.pop(0)
                        fn()
                        popped += 1
                    else:
                        break

            # ---- prelude: the three chains (h0, c0) iter-0 needs, with
            # their k-steps interleaved so each step runs as its weight tile
            # arrives instead of serializing all 24 matmuls after the last
            # arrival.  Chains 1/2 hold the two aux PSUM buffers; chain 3
            # borrows an (idle) st_pool buffer.
            pre_ps = {}

            def pre_step(which, isq, cs, n0, k0, k1):
                dst, w, bias = (qt, wq, bq_t) if isq else (kt, wk, bk_t)
                if k0 == 0:
                    if which < 2:
                        pre_ps[which] = aux_ps.tile([128, 512], F32, tag="aux", name="aux_ps_t")
                    else:
                        pre_ps[which] = st_pool.tile([128, NCHUNK], F32, tag="st", name="st_pre_t")[:, 0:512]
                ps = pre_ps[which]
                for k in range(k0, k1):
                    nc.tensor.matmul(
                        ps, w[k][cs], xt[k][:, n0 : n0 + 512],
                        start=(k == 0), stop=(k == KT - 1),
                    )
                if k1 == KT:
                    nc.vector.tensor_scalar_add(
                        out=dst[cs][:, n0 : n0 + 512], in0=ps, scalar1=bias[cs]
                    )

            pre_chains = [(True, 0, 0), (False, 0, 0), (True, 0, 512)]
            for k in range(KT):
                for which, (isq, cs, n0) in enumerate(pre_chains):
                    pre_step(which, isq, cs, n0, k, k + 1)

            # ---- attention + output projection.  The scores matmuls are
            # software-pipelined one iteration ahead of the exp so the
            # Scalar engine is never gated by the PE queue: PE order per
            # iter is [fillers, st(i+1), O(i)], with exp(i) already issued.
            seq = [
                (c, h, ms)
                for c in range(NCHUNKS)
                for h in range(HPG)
                for ms in range(MS)
            ]

            def emit_st(idx):
                c, h, ms = seq[idx]
                cs, r0 = h // 2, (h % 2) * 64
                st = st_pool.tile([128, NCHUNK], F32, tag="st", name="st_t")
                for j in range(0, NCHUNK, 512):
                    nc.tensor.matmul(
                        st[:, j : j + 512],
                        kt[cs][r0 : r0 + 64, ms * 128 : (ms + 1) * 128],
                        qt[cs][r0 : r0 + 64, c * NCHUNK + j : c * NCHUNK + j + 512],
                        start=True,
                        stop=True,
                    )
                return st

            sts = {0: emit_st(0)}
            ot = None
            for it, (chunk, h, ms) in enumerate(seq):
                if ms == 0:
                    if h == 0:
                        osb_tiles[chunk] = [
                            osb_pool.tile([128, NCHUNK], BF16, tag=f"osb{cs}", name=f"osb{cs}")
                            for cs in range(CH // 128)
                        ]
                    ot = ot_pool.tile([65, NCHUNK], F32, tag="ot", name="ot_t")
                et = et_pool.tile([128, NCHUNK], BF16, tag="et", name="et_t")
                nc.scalar.activation(
                    out=et,
                    in_=sts.pop(it),
                    func=mybir.ActivationFunctionType.Exp,
                    scale=float(1.0 / np.sqrt(DK)),
                )
                if it + 1 < len(seq):
                    sts[it + 1] = emit_st(it + 1)
                run_sched(it)
                lhsT = v4[ms][:, h * 65 : (h + 1) * 65]
                for j in range(0, NCHUNK, 512):
                    nc.tensor.matmul(
                        ot[:, j : j + 512],
                        lhsT,
                        et[:, j : j + 512],
                        start=(ms == 0),
                        stop=(ms == MS - 1),
                    )
                if ms == MS - 1 and it + 1 < len(seq):
                    normalize(chunk, h, ot)
            # ---- epilogue: drain ot on the idle ScalarE, run the reserved
            # chunk-0 groups (copies alternating DVE/ScalarE) while the
            # reciprocal chain completes, then chunk 1's projection.
            assert not quanta, len(quanta)
            oraw_last = normalize_last_a(ot)
            for i, (msl, j) in enumerate(tail_reserve):
                emit_f(0, msl, j, epi=i)
            norm_chain(NCHUNKS - 1, HPG - 1, oraw_last)
            for i in range(D // 128 * 2):
                msl, j = divmod(i, 2)
                emit_f(1, msl, j * 512, epi=10 + i)
    nc.compile()
    return nc


_NC = None


def _get_nc():
    global _NC
    if _NC is None:
        _NC = _build_bass()
    return _NC


def build_in_maps(inputs):
    x = np.asarray(inputs["x"], dtype=np.float32)
    W_Q = np.asarray(inputs["W_Q"], dtype=np.float32)
    W_K = np.asarray(inputs["W_K"], dtype=np.float32)
    W_V = np.asarray(inputs["W_V"], dtype=np.float32)
    W_O = np.asarray(inputs["W_O"], dtype=np.float32)
    b_Q = np.asarray(inputs["b_Q"], dtype=np.float32)
    b_K = np.asarray(inputs["b_K"], dtype=np.float32)
    b_V = np.asarray(inputs["b_V"], dtype=np.float32)

    in_maps = []
    for c in range(NCORES):
        b, g = divmod(c, GROUPS)
        lo = g * CH
        sl = slice(lo, lo + CH)
        wA = np.concatenate(
            [W_Q[lo : lo + 128, :].T, W_K[lo : lo + 128, :].T], axis=1
        )
        wB = np.concatenate(
            [W_Q[lo + 128 : lo + 256, :].T, W_K[lo + 128 : lo + 256, :].T],
            axis=1,
        )
        bqk = np.zeros((128, 128), dtype=np.float32)
        bqk[:, 0] = b_Q[lo : lo + 128]
        bqk[:, 1] = b_Q[lo + 128 : lo + 256]
        bqk[:, 2] = b_K[lo : lo + 128]
        bqk[:, 3] = b_K[lo + 128 : lo + 256]
        in_maps.append(
            {
                "xT": np.ascontiguousarray(x[b].T.astype(ml_dtypes.bfloat16)),
                "wA": np.ascontiguousarray(wA.astype(ml_dtypes.bfloat16)),
                "wB": np.ascontiguousarray(wB.astype(ml_dtypes.bfloat16)),
                "wv": np.ascontiguousarray(W_V[sl, :].T.astype(ml_dtypes.bfloat16)),
                "woT": np.ascontiguousarray(W_O[:, sl].T.astype(ml_dtypes.bfloat16)),
                "bqk": bqk,
                "bv": np.ascontiguousarray(b_V[sl]),
            }
        )
    return in_maps


def kernel(**inputs):
    in_maps = build_in_maps(inputs)
    nc = _get_nc()
    res = run_bass_kernel_spmd(nc, in_maps, core_ids=list(range(NCORES)))

    b_O = np.asarray(inputs["b_O"], dtype=np.float32)
    out = np.zeros((B, N, D), dtype=np.float32)
    for c in range(NCORES):
        b = c // GROUPS
        out[b] += res.results[c]["yT"].T.astype(np.float32)
    out += b_O
    return out


# revision 47
# speedup vs baseline: 1.0268x; 1.0216x over previous
# Multi-head attention (b=2, n=2048, d_model=1024, 16 heads) on 8 NeuronCores.
#
# Sharding: core c = (batch b, head-group g) with b = c//4, g = c%4.
# Each core handles 1 batch element and 4 heads (256 channels), computing a
# partial output projection; the host sums the 4 group-partials per batch and
# adds b_O.
#
# Device layout (everything oriented so no transposes are needed):
#   xT   [D, N]      = x[b].T (bf16)             rhs of Q/K proj, lhsT of V
#   Q.T/K.T [2][128, N]  2 heads per 128-row tile (cs = h//2, rows (h%2)*64).
#   V    [N, CH] natural layout (+b_v), stored per-head with an appended
#        ones column: lhsT [m, 65] so the O-matmul's PSUM row 64 accumulates
#        the softmax denominators for free.
#   S.T  [m-slice, n] per head = K_h @ Q_h.T     (K=64 contraction)
#   E.T  = exp(S.T / 8) on ScalarE (scale folded into the activation), bf16
#   O.T+sums [65, n] = [V_h | 1].T @ E.T         (accumulate over m-slices)
#   Y.T  [D, N] = woT.T @ (O.T * recip(sums)), DVE-copied to fp16, DMA.
#
# Schedule: the attention loop (128 (chunk,head,ms) iterations) is paced by
# the ScalarE exp (~1.11us per [128,1024] tile).  All projection work is
# split into ~2-matmul "quanta" and drip-fed into the per-iteration slack by
# a deadline-driven scheduler, so the PE never idles and holds its 2.4 GHz
# p-state.  Softmax normalization is split: the reciprocal chain runs right
# after each head, but the (broadcast x multiply) into osb is deferred ~4
# iterations so the PE-queue broadcast matmul never stalls the stream.
#
# Matmul operands are bf16 (fp32 PSUM accumulation); fp32r measured ~3x
# slower on HW (cold-HAM equilibrium at ~630ns per 512-row matmul).

import ml_dtypes
import numpy as np

import concourse.bass as bass
import concourse.bacc as bacc
import concourse.tile as tile
from concourse import mybir
from concourse.bass_utils import run_bass_kernel_spmd

D = 1024  # d_model
N = 2048  # sequence length
B = 2  # batch
NHEADS = 16
DK = 64
NCORES = 8
GROUPS = 4  # head-groups across cores
HPG = NHEADS // GROUPS  # 4 heads per group
CH = HPG * DK  # 256 channels per group
KT = D // 128  # 8 contraction tiles for the projections
MS = N // 128  # 16 m-slices (key dim)
NCHUNK = 1024  # n-chunk width for the attention phase
NCHUNKS = N // NCHUNK

F32 = mybir.dt.float32
F16 = mybir.dt.float16
BF16 = mybir.dt.bfloat16


def _build_bass():
    nc = bacc.Bacc()

    xT_d = nc.dram_tensor("xT", [D, N], BF16, kind="ExternalInput")
    # wA: first-needed weight columns [wq_cs0 | wk_cs0]; wB: the rest
    # [wv | wq_cs1 | wk_cs1].  bqk: bq/bk as 4 columns of a 512B-row tile
    # (single efficient DMA instead of four 4B-descriptor ones).
    wA_d = nc.dram_tensor("wA", [D, 256], BF16, kind="ExternalInput")
    wB_d = nc.dram_tensor("wB", [D, 256], BF16, kind="ExternalInput")
    wv_d = nc.dram_tensor("wv", [D, 256], BF16, kind="ExternalInput")
    woT_d = nc.dram_tensor("woT", [CH, D], BF16, kind="ExternalInput")
    bqk_d = nc.dram_tensor("bqk", [128, 128], F32, kind="ExternalInput")
    bv_d = nc.dram_tensor("bv", [CH], F32, kind="ExternalInput")
    yT_d = nc.dram_tensor("yT", [D, N], F16, kind="ExternalOutput")

    with tile.TileContext(nc) as tc:
        with (
            tc.tile_pool(name="persist", bufs=1) as persist,
            tc.tile_pool(name="ph1", bufs=1) as ph1,
            tc.tile_pool(name="et_pool", bufs=4) as et_pool,
            tc.tile_pool(name="osb_pool", bufs=2) as osb_pool,
            tc.tile_pool(name="small", bufs=2) as small,
            tc.tile_pool(name="aux_ps", bufs=2, space="PSUM") as aux_ps,
            tc.tile_pool(name="st_ps", bufs=2, space="PSUM") as st_pool,
            tc.tile_pool(name="ot_ps", bufs=1, space="PSUM") as ot_pool,
        ):
            # ---- persistent tensors ----
            qt = [persist.tile([128, N], BF16, tag=f"qt{cs}", name=f"qt{cs}") for cs in range(CH // 128)]
            kt = [persist.tile([128, N], BF16, tag=f"kt{cs}", name=f"kt{cs}") for cs in range(CH // 128)]
            v4 = [persist.tile([128, HPG * 65], BF16, tag=f"v4_{ms}", name=f"v4_{ms}") for ms in range(MS)]
            wot = [persist.tile([128, D], BF16, tag=f"wot{cs}", name=f"wot{cs}") for cs in range(CH // 128)]

            # ---- input loads.  Emission order doubles as DMA-semaphore
            # allocation order (the sem pool is small and recycled FIFO), so
            # the critical-path loads (xt chunk 0, wA) are emitted FIRST;
            # later posts recycling their sems then wait on completions we
            # need anyway.  Posting is spread across sync/scalar/gpsimd.
            xt = [ph1.tile([128, N], BF16, tag=f"xt{k}", name=f"xt{k}") for k in range(KT)]
            for k in range(KT):
                nc.sync.dma_start(out=xt[k][:, 0:NCHUNK], in_=xT_d[k * 128 : (k + 1) * 128, 0:NCHUNK])
            wA, wB = [], []
            for k in range(KT):
                t = ph1.tile([128, 256], BF16, tag=f"wA{k}", name=f"wA{k}")
                nc.scalar.dma_start(out=t, in_=wA_d[k * 128 : (k + 1) * 128, :])
                wA.append(t)
            for k in range(KT):
                t = ph1.tile([128, 256], BF16, tag=f"wB{k}", name=f"wB{k}")
                nc.scalar.dma_start(out=t, in_=wB_d[k * 128 : (k + 1) * 128, :])
                wB.append(t)
            for k in range(KT):
                nc.sync.dma_start(out=xt[k][:, NCHUNK:N], in_=xT_d[k * 128 : (k + 1) * 128, NCHUNK:N])
            # wq[k][cs], wk[k][cs] views into the packed tiles
            wq = [(wA[k][:, 0:128], wB[k][:, 0:128]) for k in range(KT)]
            wk = [(wA[k][:, 128:256], wB[k][:, 128:256]) for k in range(KT)]
            # gpsimd queue: constants, V weights (needed from iter 0), then
            # late-needed weights/biases
            for ms in range(MS):
                v4v = v4[ms].rearrange("p (h c) -> p h c", c=65)
                nc.gpsimd.memset(v4v[:, :, 64:65], 1.0)
            bqk = ph1.tile([128, 128], F32, tag="bqk", name="bqk")
            nc.gpsimd.dma_start(out=bqk, in_=bqk_d[:, :])
            bq_t = [bqk[:, 0:1], bqk[:, 1:2]]
            bk_t = [bqk[:, 2:3], bqk[:, 3:4]]
            bvb = ph1.tile([128, CH], F32, tag="bvb", name="bvb")
            bv_ap = bv_d[None, :]
            nc.gpsimd.dma_start(
                out=bvb,
                in_=bass.AP(tensor=bv_ap.tensor, offset=bv_ap.offset, ap=[[0, 128]] + list(bv_ap.ap[1:])),
            )
            wv = []
            for k in range(KT):
                t = ph1.tile([128, 256], BF16, tag=f"wv{k}", name=f"wv{k}")
                nc.gpsimd.dma_start(out=t, in_=wv_d[k * 128 : (k + 1) * 128, :])
                wv.append(t)
            for cs in range(CH // 128):
                nc.gpsimd.dma_start(out=wot[cs], in_=woT_d[cs * 128 : (cs + 1) * 128, :])

            osb_tiles = {}

            # ---- chain emitters, split into ~2-matmul quanta ----
            def qk_chain_quanta(isq, cs, n0):
                dst, w, bias = (qt, wq, bq_t) if isq else (kt, wk, bk_t)
                st_ = {}

                def step(k0, k1):
                    if k0 == 0:
                        st_["ps"] = aux_ps.tile([128, 512], F32, tag="aux", name="aux_ps_t")
                    ps = st_["ps"]
                    for k in range(k0, k1):
                        nc.tensor.matmul(
                            ps,
                            w[k][cs],
                            xt[k][:, n0 : n0 + 512],
                            start=(k == 0),
                            stop=(k == KT - 1),
                        )
                    if k1 == KT:
                        nc.vector.tensor_scalar_add(
                            out=dst[cs][:, n0 : n0 + 512], in0=ps, scalar1=bias[cs]
                        )

                return [lambda a=a, b=b: step(a, b) for a, b in ((0, 2), (2, 4), (4, 6), (6, 8))]

            def v_chain_quanta(ms):
                st_ = {}

                def step(k0, k1):
                    if k0 == 0:
                        st_["ps"] = aux_ps.tile([128, 512], F32, tag="aux", name="aux_ps_t")
                    ps = st_["ps"]
                    for k in range(k0, k1):
                        nc.tensor.matmul(
                            ps[:, 0:CH],
                            xt[k][:, ms * 128 : (ms + 1) * 128],
                            wv[k],
                            start=(k == 0),
                            stop=(k == KT - 1),
                        )
                    if k1 == KT:
                        v4v = v4[ms].rearrange("p (h c) -> p h c", c=65)
                        nc.vector.tensor_add(
                            out=v4v[:, :, 0:64],
                            in0=ps[:, 0:CH].rearrange("p (h c) -> p h c", c=64),
                            in1=bvb.rearrange("p (h c) -> p h c", c=64),
                        )

                return [lambda a=a, b=b: step(a, b) for a, b in ((0, 2), (2, 4), (4, 6), (6, 8))]

            def emit_f(chunk, msl, j, epi=None):
                # epi: epilogue slot index — rotates PSUM over the idle
                # st_pool as well as aux, alternates the drain copy between
                # ScalarE and DVE, and the yT DMA between sync and gpsimd.
                n0 = chunk * NCHUNK
                osb = osb_tiles[chunk]
                if epi is not None and epi % 2 == 0:
                    yp = st_pool.tile([128, NCHUNK], F32, tag="st", name="st_yt_t")[:, 0:512]
                else:
                    yp = aux_ps.tile([128, 512], F32, tag="aux", name="aux_yt_t")
                for cs in range(CH // 128):
                    nc.tensor.matmul(
                        yp,
                        wot[cs][:, msl * 128 : (msl + 1) * 128],
                        osb[cs][:, j : j + 512],
                        start=(cs == 0),
                        stop=(cs == CH // 128 - 1),
                    )
                ysb = small.tile([128, 512], F16, tag="ysb", name="ysb_t", bufs=4)
                if epi is not None and epi % 2 == 1:
                    nc.scalar.copy(out=ysb, in_=yp)
                else:
                    nc.vector.tensor_copy(out=ysb, in_=yp)
                nc.sync.dma_start(
                    out=yT_d[msl * 128 : (msl + 1) * 128, n0 + j : n0 + j + 512],
                    in_=ysb,
                )

            # softmax normalization, entirely off the PE: drain ot, reshuffle
            # the denominator row to [128, 8] via a tiny DMA, reciprocal,
            # flatten back, partition-broadcast on the (idle) GpSimd, and
            # scale the O.T rows into osb on the DVE.
            def norm_chain(chunk, h, oraw):
                # [8, 128] staging keeps both reshuffle DMAs at 8 fat
                # descriptors instead of 128 tiny ones (~0.9us vs ~2.4us).
                cs, r0 = h // 2, (h % 2) * 64
                rcin = small.tile([8, 128], F32, tag="rcin", name="rcin_t")
                nc.gpsimd.dma_start(out=rcin, in_=oraw[64:65, :])
                rc = small.tile([8, 128], F32, tag="rc", name="rc_t")
                nc.vector.reciprocal(out=rc, in_=rcin)
                rflat = small.tile([1, NCHUNK], F32, tag="rflat", name="rflat_t")
                nc.gpsimd.dma_start(out=rflat, in_=rc)
                rb = small.tile([128, NCHUNK], F32, tag="rb", name="rb_t")
                nc.gpsimd.partition_broadcast(rb, rflat)
                nc.vector.tensor_mul(
                    out=osb_tiles[chunk][cs][r0 : r0 + 64, :],
                    in0=oraw[0:64, :],
                    in1=rb[0:64, :],
                )

            def normalize(chunk, h, ot):
                oraw = small.tile([65, NCHUNK], F32, tag="oraw", name="oraw_t")
                nc.vector.tensor_copy(out=oraw, in_=ot)
                norm_chain(chunk, h, oraw)

            # Final head's normalization, split so the reserved output-
            # projection groups can be emitted in between: part A drains ot
            # on the (now idle) ScalarE; part B runs the reciprocal chain.
            # Nothing in the reserved groups then waits on the chain through
            # the in-order DVE/scalar queues.
            def normalize_last_a(ot):
                oraw = small.tile([65, NCHUNK], F32, tag="oraw", name="oraw_t")
                nc.scalar.copy(out=oraw, in_=ot)
                return oraw

            # ---- quantum schedule.  Chains are kept CONTIGUOUS in pop
            # order (only consecutive chains ever co-occupy the 2-buffer
            # aux PSUM pool); each chain gets a start deadline and its
            # quanta inherit start+qi.  Negative deadlines pop at iter 0.
            chains = []  # (start_deadline, ready_iter, [quanta...])
            for ms in range(0, MS):  # V(ms) must close before O at iter ms
                chains.append((ms - 3, 0, v_chain_quanta(ms)))
            for i, n0 in enumerate((512, 1024, 1536)):  # kt cs0, st @ iter 4i+4
                chains.append((4 * i + 1, 0, qk_chain_quanta(False, 0, n0)))
            for i, n0 in enumerate((0, 512)):  # qt cs1 chunk0 (h2c0 @ iter 32)
                chains.append((24 + 2 * i, 0, qk_chain_quanta(True, 1, n0)))
            for i, n0 in enumerate((0, 512, 1024, 1536)):  # kt cs1 (h2c0)
                rdy = 12 if n0 >= 1024 else 0
                chains.append((28 + 4 * i, rdy, qk_chain_quanta(False, 1, n0)))
            for i, n0 in enumerate((1024, 1536)):  # qt cs0 chunk1 (h0c1 @ 64)
                chains.append((48 + 4 * i, 12, qk_chain_quanta(True, 0, n0)))
            for i, n0 in enumerate((1024, 1536)):  # qt cs1 chunk1 (h2c1 @ 96)
                chains.append((58 + 4 * i, 12, qk_chain_quanta(True, 1, n0)))
            # output projection for chunk 0: fillers once chunk-0 osb is
            # fully normalized (~iter 68); hold back the last 10 groups to
            # keep the PE warm through the final reciprocal chain.
            fgroups = [(msl, j) for msl in range(D // 128) for j in range(0, NCHUNK, 512)]
            for i, (msl, j) in enumerate(fgroups[:2]):
                chains.append((69 + 4 * i, 69, [lambda m=msl, jj=j: emit_f(0, m, jj)]))
            tail_reserve = fgroups[2:]
            chains.sort(key=lambda c: c[0])
            quanta = [
                (start + qi, rdy, fn)
                for start, rdy, qs in chains
                for qi, fn in enumerate(qs)
            ]

            def run_sched(t):
                popped = 0
                while quanta:
                    dl, rdy, fn = quanta[0]
                    due = dl <= t or any(q[0] <= t for q in quanta[1:6])
                    if due or (popped == 0 and rdy <= t):
                        quanta.pop(0)
                        fn()
                        popped += 1
                    else:
                        break

            # ---- prelude: the three chains (h0, c0) iter-0 needs, with
            # their k-steps interleaved so each step runs as its weight tile
            # arrives instead of serializing all 24 matmuls after the last
            # arrival.  Chains 1/2 hold the two aux PSUM buffers; chain 3
            # borrows an (idle) st_pool buffer.
            pre_ps = {}

            def pre_step(which, isq, cs, n0, k0, k1):
                dst, w, bias = (qt, wq, bq_t) if isq else (kt, wk, bk_t)
                if k0 == 0:
                    if which < 2:
                        pre_ps[which] = aux_ps.tile([128, 512], F32, tag="aux", name="aux_ps_t")
                    else:
                        pre_ps[which] = st_pool.tile([128, NCHUNK], F32, tag="st", name="st_pre_t")[:, 0:512]
                ps = pre_ps[which]
                for k in range(k0, k1):
                    nc.tensor.matmul(
                        ps, w[k][cs], xt[k][:, n0 : n0 + 512],
                        start=(k == 0), stop=(k == KT - 1),
                    )
                if k1 == KT:
                    nc.vector.tensor_scalar_add(
                        out=dst[cs][:, n0 : n0 + 512], in0=ps, scalar1=bias[cs]
                    )

            pre_chains = [(True, 0, 0), (False, 0, 0), (True, 0, 512)]
            for k in range(KT):
                for which, (isq, cs, n0) in enumerate(pre_chains):
                    pre_step(which, isq, cs, n0, k, k + 1)

            # ---- attention + output projection.  The scores matmuls are
            # software-pipelined one iteration ahead of the exp so the
            # Scalar engine is never gated by the PE queue: PE order per
            # iter is [fillers, st(i+1), O(i)], with exp(i) already issued.
            seq = [
                (c, h, ms)
                for c in range(NCHUNKS)
                for h in range(HPG)
                for ms in range(MS)
            ]

            def emit_st(idx):
                c, h, ms = seq[idx]
                cs, r0 = h // 2, (h % 2) * 64
                st = st_pool.tile([128, NCHUNK], F32, tag="st", name="st_t")
                for j in range(0, NCHUNK, 512):
                    nc.tensor.matmul(
                        st[:, j : j + 512],
                        kt[cs][r0 : r0 + 64, ms * 128 : (ms + 1) * 128],
                        qt[cs][r0 : r0 + 64, c * NCHUNK + j : c * NCHUNK + j + 512],
                        start=True,
                        stop=True,
                    )
                return st

            sts = {0: emit_st(0)}
            ot = None
            for it, (chunk, h, ms) in enumerate(seq):
                if ms == 0:
                    if h == 0:
                        osb_tiles[chunk] = [
                            osb_pool.tile([128, NCHUNK], BF16, tag=f"osb{cs}", name=f"osb{cs}")
                            for cs in range(CH // 128)
                        ]
                    ot = ot_pool.tile([65, NCHUNK], F32, tag="ot", name="ot_t")
                et = et_pool.tile([128, NCHUNK], BF16, tag="et", name="et_t")
                nc.scalar.activation(
                    out=et,
                    in_=sts.pop(it),
                    func=mybir.ActivationFunctionType.Exp,
                    scale=float(1.0 / np.sqrt(DK)),
                )
                if it + 1 < len(seq):
                    sts[it + 1] = emit_st(it + 1)
                run_sched(it)
                lhsT = v4[ms][:, h * 65 : (h + 1) * 65]
                for j in range(0, NCHUNK, 512):
                    nc.tensor.matmul(
                        ot[:, j : j + 512],
                        lhsT,
                        et[:, j : j + 512],
                        start=(ms == 0),
                        stop=(ms == MS - 1),
                    )
                if ms == MS - 1 and it + 1 < len(seq):
                    normalize(chunk, h, ot)
            # ---- epilogue: drain ot on the idle ScalarE, run the reserved
            # chunk-0 groups (copies alternating DVE/ScalarE) while the
            # reciprocal chain completes, then chunk 1's projection.
            assert not quanta, len(quanta)
            oraw_last = normalize_last_a(ot)
            for i, (msl, j) in enumerate(tail_reserve):
                emit_f(0, msl, j, epi=i)
            norm_chain(NCHUNKS - 1, HPG - 1, oraw_last)
            for i in range(D // 128 * 2):
                msl, j = divmod(i, 2)
                emit_f(1, msl, j * 512, epi=10 + i)
    nc.compile()
    return nc


_NC = None


def _get_nc():
    global _NC
    if _NC is None:
        _NC = _build_bass()
    return _NC


def build_in_maps(inputs):
    x = np.asarray(inputs["x"], dtype=np.float32)
    W_Q = np.asarray(inputs["W_Q"], dtype=np.float32)
    W_K = np.asarray(inputs["W_K"], dtype=np.float32)
    W_V = np.asarray(inputs["W_V"], dtype=np.float32)
    W_O = np.asarray(inputs["W_O"], dtype=np.float32)
    b_Q = np.asarray(inputs["b_Q"], dtype=np.float32)
    b_K = np.asarray(inputs["b_K"], dtype=np.float32)
    b_V = np.asarray(inputs["b_V"], dtype=np.float32)

    in_maps = []
    for c in range(NCORES):
        b, g = divmod(c, GROUPS)
        lo = g * CH
        sl = slice(lo, lo + CH)
        wA = np.concatenate(
            [W_Q[lo : lo + 128, :].T, W_K[lo : lo + 128, :].T], axis=1
        )
        wB = np.concatenate(
            [W_Q[lo + 128 : lo + 256, :].T, W_K[lo + 128 : lo + 256, :].T],
            axis=1,
        )
        bqk = np.zeros((128, 128), dtype=np.float32)
        bqk[:, 0] = b_Q[lo : lo + 128]
        bqk[:, 1] = b_Q[lo + 128 : lo + 256]
        bqk[:, 2] = b_K[lo : lo + 128]
        bqk[:, 3] = b_K[lo + 128 : lo + 256]
        in_maps.append(
            {
                "xT": np.ascontiguousarray(x[b].T.astype(ml_dtypes.bfloat16)),
                "wA": np.ascontiguousarray(wA.astype(ml_dtypes.bfloat16)),
                "wB": np.ascontiguousarray(wB.astype(ml_dtypes.bfloat16)),
                "wv": np.ascontiguousarray(W_V[sl, :].T.astype(ml_dtypes.bfloat16)),
                "woT": np.ascontiguousarray(W_O[:, sl].T.astype(ml_dtypes.bfloat16)),
                "bqk": bqk,
                "bv": np.ascontiguousarray(b_V[sl]),
            }
        )
    return in_maps


def kernel(**inputs):
    in_maps = build_in_maps(inputs)
    nc = _get_nc()
    res = run_bass_kernel_spmd(nc, in_maps, core_ids=list(range(NCORES)))

    b_O = np.asarray(inputs["b_O"], dtype=np.float32)
    out = np.zeros((B, N, D), dtype=np.float32)
    for c in range(NCORES):
        b = c // GROUPS
        out[b] += res.results[c]["yT"].T.astype(np.float32)
    out += b_O
    return out
